# revision 1
# baseline (speedup 1.0000x reference)
"""Multi-head attention (B=2, S=2048, D=1024, H=16 heads, causal) on 8 TRN2
NeuronCores, head-parallel: each core computes 2 heads' Q/K/V projections,
attention, and a partial output projection (its 128-row slice of Wo); the
host sums the 8 partial outputs.

Per-core layout (matmul operands bf16, fp32 PSUM accumulation):
  - xt [128,8,8,512]     x^T pre-tiled on host as [partition, strip, k, col]
  - wq/wk/wv [128,8,128] per-core column slice of Wq/Wk/Wv, pre-tiled
  - wo [128, 1024]       per-core row slice of Wo
  - msk [128, 896]       sliding causal mask: msk[i, c] = 1 if c-384 >= i
  - idn [128, 128]       identity for PE transposes of the V projection
  QT/KT are produced transposed [128 = 2 heads x 64 head dims, 4096 tokens];
  V is produced natural per (b, h, kv-tile) as [128 kv, 64] with an appended
  ones column so the attention matmul also accumulates softmax denominators
  (row 64 of the [65, 512] PSUM output).

Emission is interleaved at "unit" granularity: while strip g's attention
(ACT-heavy) is emitted, strip g+1's projection matmuls and strip g-1's
output-projection matmuls are interspersed so the PE never idles long
enough for the HAM clock gate to re-throttle.
"""

import numpy as np
import ml_dtypes
from contextlib import ExitStack

import concourse.bass as bass
import concourse.bacc as bacc
import concourse.tile as tile
import concourse.mybir as mybir
from concourse.bass_utils import run_bass_kernel_spmd

BF16 = mybir.dt.bfloat16
F32 = mybir.dt.float32
NPBF16 = ml_dtypes.bfloat16

D = 1024          # model dim
B = 2
S = 2048
NT = B * S        # 4096 flattened tokens
HD = 64           # head dim
H = 16            # total heads
NCORES = 8
HLOC = H // NCORES  # 2 heads per core
CW = HLOC * HD      # 128 local columns
QSTRIP = 512
NSTRIP = NT // QSTRIP  # 8 strips
KT_TILES = S // 128    # 16 kv tiles per batch


def _interleave(main, fill):
    """Emit main units with fill units spread proportionally between them."""
    n, m = len(main), len(fill)
    if n == 0:
        for u in fill:
            u()
        return
    fi = 0
    for i, u in enumerate(main):
        u()
        tgt = ((i + 1) * m) // n
        while fi < tgt:
            fill[fi]()
            fi += 1
    while fi < m:
        fill[fi]()
        fi += 1


def _build_kernel(ctx: ExitStack, tc: tile.TileContext):
    nc = tc.nc
    # pre-arranged on host: xt[p, strip, k, col], w*[p, k, col]
    xt = nc.dram_tensor("xt", [128, NSTRIP, 8, QSTRIP], BF16,
                        kind="ExternalInput").ap()
    wq = nc.dram_tensor("wq", [128, 8, CW], BF16, kind="ExternalInput").ap()
    wk = nc.dram_tensor("wk", [128, 8, CW], BF16, kind="ExternalInput").ap()
    wv = nc.dram_tensor("wv", [128, 8, CW], BF16, kind="ExternalInput").ap()
    wo = nc.dram_tensor("wo", [CW, D], BF16, kind="ExternalInput").ap()
    msk = nc.dram_tensor("msk", [128, 896], BF16, kind="ExternalInput").ap()
    idn = nc.dram_tensor("idn", [128, 128], BF16, kind="ExternalInput").ap()
    out = nc.dram_tensor("out", [NT, D], F32, kind="ExternalOutput").ap()

    singles = ctx.enter_context(tc.tile_pool(name="singles", bufs=1))
    sbp = ctx.enter_context(tc.tile_pool(name="sbp", bufs=3))
    expp = ctx.enter_context(tc.tile_pool(name="expp", bufs=8))
    outp = ctx.enter_context(tc.tile_pool(name="outp", bufs=3))
    psM = ctx.enter_context(tc.tile_pool(name="psM", bufs=2, space="PSUM"))
    psS = ctx.enter_context(tc.tile_pool(name="psS", bufs=2, space="PSUM"))
    psV = ctx.enter_context(tc.tile_pool(name="psV", bufs=2, space="PSUM"))
    drp = ctx.enter_context(tc.tile_pool(name="drp", bufs=2, space="DRAM"))

    # --- staging: small weights first, then xT strip-major so strip 0's
    # projection can start ~2us in while later strips stream in behind it.
    w_sb = {}
    xt_sb = singles.tile([128, 8, NT], BF16)

    def load_w(name, w):
        t = singles.tile([128, 8, CW], BF16, tag=f"w{name}", name=f"w_{name}")
        nc.gpsimd.dma_start(out=t, in_=w)
        w_sb[name] = t

    def load_xt(g):
        gs = g * QSTRIP
        nc.sync.dma_start(out=xt_sb[:, :, gs:gs + QSTRIP], in_=xt[:, g, :, :])

    load_w("q", wq)
    # strip 0 split across both DMA queues to cut time-to-first-matmul
    nc.sync.dma_start(out=xt_sb[:, 0:4, 0:QSTRIP], in_=xt[:, 0, 0:4, :])
    nc.gpsimd.dma_start(out=xt_sb[:, 4:8, 0:QSTRIP], in_=xt[:, 0, 4:8, :])
    load_xt(1)
    load_w("k", wk)
    load_w("v", wv)
    msk_sb = singles.tile([128, 896], BF16)
    nc.gpsimd.dma_start(out=msk_sb, in_=msk)
    idn_sb = singles.tile([128, 128], BF16)
    nc.gpsimd.dma_start(out=idn_sb, in_=idn)
    wo_sb = singles.tile([128, D], BF16)
    nc.gpsimd.dma_start(out=wo_sb, in_=wo)
    for g in range(2, NSTRIP):
        load_xt(g)

    qt_sb = singles.tile([128, NT], BF16)
    kt_sb = singles.tile([128, NT], BF16)
    v_sb = singles.tile([128, B * HLOC * KT_TILES, HD + 1], BF16)
    nc.vector.memset(v_sb[:, :, HD:HD + 1], 1.0)

    avf = {}  # strip -> assembled [128, 512] bf16 avT tile (both heads)

    def proj_units(g):
        gs = g * QSTRIP
        st = {}

        def qk_mm(name, lo, hi, first, last, dst):
            def u():
                if first:
                    st[name] = psM.tile([128, QSTRIP], F32, tag="mm", name=f"ps_{name}")
                ps = st[name]
                for k in range(lo, hi):
                    nc.tensor.matmul(
                        ps, lhsT=w_sb[name][:, k, :],
                        rhs=xt_sb[:, k, gs:gs + QSTRIP],
                        start=(k == 0), stop=(k == 7))
                if last:
                    nc.vector.tensor_copy(dst[:, gs:gs + QSTRIP], ps)
            return u

        groups = {"q": [qk_mm("q", 0, 4, True, False, qt_sb),
                        qk_mm("q", 4, 8, False, True, qt_sb)],
                  "k": [qk_mm("k", 0, 4, True, False, kt_sb),
                        qk_mm("k", 4, 8, False, True, kt_sb)]}

        b, j = divmod(g, 4)

        def vt_mm(lo, hi, first, last):
            def u():
                if first:
                    st["v"] = psM.tile([128, QSTRIP], F32, tag="mm", name="ps_v")
                ps = st["v"]
                for k in range(lo, hi):
                    nc.tensor.matmul(
                        ps, lhsT=w_sb["v"][:, k, :],
                        rhs=xt_sb[:, k, gs:gs + QSTRIP],
                        start=(k == 0), stop=(k == 7))
                if last:
                    st["vt"] = sbp.tile([128, QSTRIP], BF16, tag="vt",
                                        name="vt_sb")
                    nc.vector.tensor_copy(st["vt"], ps)
            return u

        def v_tr():
            def u():
                tp = psM.tile([128, 4, 128], BF16, tag="mm", name="tp_ps")
                for tt in range(4):
                    nc.tensor.transpose(
                        tp[:, tt, :], st["vt"][:, tt * 128:(tt + 1) * 128],
                        idn_sb)
                for tt in range(4):
                    idx = b * HLOC * KT_TILES + 4 * j + tt
                    nc.vector.tensor_copy(
                        v_sb[:, idx:idx + KT_TILES + 1:KT_TILES, 0:HD],
                        tp[:, tt, :].rearrange("p (h d) -> p h d", h=2))
            return u

        groups["v"] = [vt_mm(0, 4, True, False), vt_mm(4, 8, False, True),
                       v_tr()]
        return groups

    def attn_units(g, head_seq=False):
        b, j = divmod(g, 4)
        units = []
        st = {}

        def mk_pair(h, p):
            def u():
                if p == 0:
                    if h == 0 and g not in avf:
                        avf[g] = sbp.tile([128, QSTRIP], BF16, tag="avf", name="avf")
                    st[f"av{h}"] = psV.tile([HD + 1, QSTRIP], F32, tag="av",
                                            name="av_ps")
                av_ps = st[f"av{h}"]
                hp = h * HD
                ntl = 4 * (j + 1)
                ts = (2 * p, 2 * p + 1)
                q0s = [max(0, 128 * (t - 4 * j)) for t in ts]
                sc_ps = psS.tile([128, 2, QSTRIP], F32, tag="sc", name="sc_ps")
                for i, t in enumerate(ts):
                    nc.tensor.matmul(
                        sc_ps[:, i, q0s[i]:],
                        lhsT=kt_sb[hp:hp + HD,
                                   b * S + t * 128: b * S + (t + 1) * 128],
                        rhs=qt_sb[hp:hp + HD,
                                  b * S + j * QSTRIP + q0s[i]:
                                  b * S + (j + 1) * QSTRIP],
                        start=True, stop=True)
                pexp = expp.tile([128, 2, QSTRIP], BF16, tag="pexp", name="pexp")
                if q0s[0] == q0s[1]:
                    # one wide exp over both kv tiles
                    nc.scalar.activation(
                        pexp[:, :, q0s[0]:], sc_ps[:, :, q0s[0]:],
                        mybir.ActivationFunctionType.Exp, scale=0.125)
                else:  # diagonal pair: exact valid ranges per tile
                    for i in range(2):
                        nc.scalar.activation(
                            pexp[:, i, q0s[i]:], sc_ps[:, i, q0s[i]:],
                            mybir.ActivationFunctionType.Exp, scale=0.125)
                for i, t in enumerate(ts):
                    r = t - 4 * j
                    if r >= 0:  # triangular mask on the diagonal 128-block
                        nc.vector.tensor_mul(
                            pexp[:, i, q0s[i]:q0s[i] + 128],
                            pexp[:, i, q0s[i]:q0s[i] + 128],
                            msk_sb[:, 384:512])
                    idx = (b * HLOC + h) * KT_TILES + t
                    nc.tensor.matmul(
                        av_ps[:, q0s[i]:], lhsT=v_sb[:, idx, :],
                        rhs=pexp[:, i, q0s[i]:],
                        start=(t == 0), stop=(t == ntl - 1))
            return u

        def mk_norm(h):
            def u():
                av_ps = st[f"av{h}"]
                s_sb = sbp.tile([HD + 1, QSTRIP], F32, tag="s", name="s_sb")
                nc.vector.tensor_copy(s_sb[HD:HD + 1, :], av_ps[HD:HD + 1, :])
                s_dr = drp.tile([1, QSTRIP], F32, tag="sdr")
                nc.sync.dma_start(out=s_dr, in_=s_sb[HD:HD + 1, :])
                rb = sbp.tile([HD, QSTRIP], F32, tag="rb")
                nc.sync.dma_start(
                    out=rb, in_=s_dr[0, :].partition_broadcast(HD))
                nc.vector.reciprocal_approx_fast(rb, rb)
                if h == 0:  # partitions 0-63: lane-aligned direct write
                    nc.vector.tensor_mul(avf[g][0:HD, :], av_ps[0:HD, :], rb)
                else:
                    avh = sbp.tile([HD, QSTRIP], BF16, tag="avh")
                    nc.vector.tensor_mul(avh, av_ps[0:HD, :], rb)
                    nc.sync.dma_start(out=avf[g][HD:2 * HD, :], in_=avh)
            return u

        if head_seq:
            for h in range(HLOC):
                for p in range(2 * (j + 1)):
                    units.append(mk_pair(h, p))
                units.append(mk_norm(h))
        else:
            for p in range(2 * (j + 1)):
                for h in range(HLOC):
                    units.append(mk_pair(h, p))
            units.append(mk_norm(0))
            units.append(mk_norm(1))
        return units

    def out_units(g):
        gs = g * QSTRIP
        units = []

        def mk(tt):
            def u():
                ob = outp.tile([128, D], F32, tag="ob")
                for n in range(2):
                    op_ps = psM.tile([128, 512], F32, tag="mm", name="op_ps")
                    nc.tensor.matmul(
                        op_ps, lhsT=avf[g][:, tt * 128:(tt + 1) * 128],
                        rhs=wo_sb[:, n * 512:(n + 1) * 512],
                        start=True, stop=True)
                    nc.vector.tensor_copy(ob[:, n * 512:(n + 1) * 512], op_ps)
                nc.sync.dma_start(
                    out=out[gs + tt * 128: gs + (tt + 1) * 128, :], in_=ob)
            return u
        for tt in range(4):
            units.append(mk(tt))
        return units

    # strip order: b0 ascending then b1 descending (short strip last).
    # fill[g] lists (strip, groups) of projection work + out strips whose
    # units are interleaved into attn(g)'s ACT-bound stretch.
    order = [0, 1, 2, 3, 7, 6, 5, 4]
    pu = {g: proj_units(g) for g in range(NSTRIP)}

    def pf(g, keys):
        return [u for k in keys for u in pu[g][k]]

    for u in pf(0, "qkv"):
        u()
    def _merge(a, b):
        out = []
        n, m = len(a), len(b)
        bi = 0
        for i, u in enumerate(a):
            out.append(u)
            tgt = ((i + 1) * m) // n
            while bi < tgt:
                out.append(b[bi])
                bi += 1
        out.extend(b[bi:])
        return out

    fill_sched = {
        0: lambda: pf(1, "qkv"),
        1: lambda: pf(2, "qkv") + out_units(0),
        2: lambda: pf(3, "qkv") + pf(4, "kv") + out_units(1),
        3: lambda: pf(5, "kv") + pf(6, "kv") + pf(7, "qkv") + out_units(2),
        7: lambda: out_units(3) + pf(6, "q"),
        6: lambda: out_units(7) + pf(5, "q") + pf(4, "q"),
    }
    # dedicated (non-pooled) avT tensors for the merged strips 5 and 4:
    # no pool-slot reuse -> no WAR hazard across the merged window
    avf[5] = singles.tile([128, QSTRIP], BF16, tag="avf5", name="avf5")
    avf[4] = singles.tile([128, QSTRIP], BF16, tag="avf4", name="avf4")
    for g in [0, 1, 2, 3, 7, 6]:
        _interleave(attn_units(g), fill_sched[g]())
    merged = _merge(attn_units(5, head_seq=True), attn_units(4, head_seq=True))
    _interleave(merged, out_units(6))
    for u in out_units(5) + out_units(4):
        u()


_CACHED_NC = None


def build_module():
    global _CACHED_NC
    if _CACHED_NC is None:
        nc = bacc.Bacc("TRN2", debug=False)
        with tile.TileContext(nc) as tc:
            with ExitStack() as ctx:
                _build_kernel(ctx, tc)
        nc.compile()
        _CACHED_NC = nc
    return _CACHED_NC


def make_in_maps(x, Wq, Wk, Wv, Wo):
    x = np.asarray(x, np.float32)
    xT = x.reshape(NT, D).T.astype(NPBF16)          # [D, NT]
    # device layout [p, strip, k, col]: row d = k*128 + p
    xT = np.ascontiguousarray(
        xT.reshape(8, 128, NSTRIP, QSTRIP).transpose(1, 2, 0, 3))
    # sliding causal mask: keep (c - 384) >= i
    i = np.arange(128)[:, None]
    c = np.arange(896)[None, :]
    msk = ((c - 384) >= i).astype(NPBF16)
    in_maps = []
    for core in range(NCORES):
        cs = slice(core * CW, (core + 1) * CW)
        def warr(W):  # [D, CW] -> [p, k, col] with d = k*128 + p
            a = np.asarray(W, np.float32)[:, cs].astype(NPBF16)
            return np.ascontiguousarray(
                a.reshape(8, 128, CW).transpose(1, 0, 2))
        in_maps.append({
            "xt": xT,
            "wq": warr(Wq),
            "wk": warr(Wk),
            "wv": warr(Wv),
            "wo": np.ascontiguousarray(np.asarray(Wo, np.float32)[cs, :]).astype(NPBF16),
            "msk": msk,
            "idn": np.eye(128, dtype=NPBF16),
        })
    return in_maps


def kernel(x, Wq, bq, Wk, bk, Wv, bv, Wo, bo):
    for b_ in (bq, bk, bv, bo):
        assert np.count_nonzero(np.asarray(b_)) == 0, "nonzero biases unsupported"
    nc = build_module()
    in_maps = make_in_maps(x, Wq, Wk, Wv, Wo)
    res = run_bass_kernel_spmd(nc, in_maps, core_ids=list(range(NCORES)))
    partials = [res.results[c]["out"] for c in range(NCORES)]
    total = np.sum(np.stack(partials, 0), axis=0, dtype=np.float32)
    return total.reshape(B, S, D)



# revision 8
# speedup vs baseline: 1.2155x; 1.2155x over previous
"""Multi-head attention (B=2, S=2048, D=1024, H=16 heads, causal) on 8 TRN2
NeuronCores, head-parallel: each core computes 2 heads' Q/K/V projections,
attention, and a partial output projection (its 128-row slice of Wo); the
host sums the 8 partial outputs (bf16 partials, fp32 sum).

Per-core layout (matmul operands bf16, fp32 PSUM accumulation):
  - xt [128,8,8,512]     x^T pre-tiled on host as [partition, strip, k, col]
  - wq/wk/wv [128,8,128] per-core column slice of Wq/Wk/Wv, pre-tiled
  - wo [128, 1024]       per-core row slice of Wo
  - msk [128, 2, 128]    upper-triangular keep mask, duplicated for 2 heads
  - idn [128, 128]       identity for PE transposes of the V projection
  QT/KT are produced transposed [128 = 2 heads x 64 head dims, 4096 tokens];
  V is stored per (b, h, kv-tile) as [128 kv, 64] plus an appended ones
  column so the attention matmul also accumulates softmax denominators
  (row 64 of the [65, 512] PSUM output).

Attention is emitted at kv-tile granularity: for each 128-kv tile the two
heads' score matmuls (K=64) are adjacent so they run concurrently in the
upper/lower halves of the PE array (row tiling); one 1024-wide exp covers
both heads. Projection/output-projection units are interleaved between
tile units so the PE never idles long enough to re-throttle.
"""

import numpy as np
import ml_dtypes
from contextlib import ExitStack

import concourse.bass as bass
import concourse.bacc as bacc
import concourse.tile as tile
import concourse.mybir as mybir
from concourse.bass_utils import run_bass_kernel_spmd

BF16 = mybir.dt.bfloat16
F32 = mybir.dt.float32
NPBF16 = ml_dtypes.bfloat16

D = 1024          # model dim
B = 2
S = 2048
NT = B * S        # 4096 flattened tokens
HD = 64           # head dim
H = 16            # total heads
NCORES = 8
HLOC = H // NCORES  # 2 heads per core
CW = HLOC * HD      # 128 local columns
QSTRIP = 512
NSTRIP = NT // QSTRIP  # 8 strips
KT_TILES = S // 128    # 16 kv tiles per batch


def _interleave(main, fill):
    """Emit main units with fill units spread proportionally between them."""
    n, m = len(main), len(fill)
    if n == 0:
        for u in fill:
            u()
        return
    fi = 0
    for i, u in enumerate(main):
        u()
        tgt = ((i + 1) * m) // n
        while fi < tgt:
            fill[fi]()
            fi += 1
    while fi < m:
        fill[fi]()
        fi += 1


def _build_kernel(ctx: ExitStack, tc: tile.TileContext):
    nc = tc.nc
    # pre-arranged on host: xt[p, strip, k, col], w*[p, k, col]
    xt = nc.dram_tensor("xt", [128, NSTRIP, 8, QSTRIP], BF16,
                        kind="ExternalInput").ap()
    wq = nc.dram_tensor("wq", [128, 8, CW], BF16, kind="ExternalInput").ap()
    wk = nc.dram_tensor("wk", [128, 8, CW], BF16, kind="ExternalInput").ap()
    wv = nc.dram_tensor("wv", [128, 8, CW], BF16, kind="ExternalInput").ap()
    wo = nc.dram_tensor("wo", [CW, D], BF16, kind="ExternalInput").ap()
    msk = nc.dram_tensor("msk", [128, 2, 128], BF16, kind="ExternalInput").ap()
    idn = nc.dram_tensor("idn", [128, 128], BF16, kind="ExternalInput").ap()
    out = nc.dram_tensor("out", [NT, D], BF16, kind="ExternalOutput").ap()

    singles = ctx.enter_context(tc.tile_pool(name="singles", bufs=1))
    sbp = ctx.enter_context(tc.tile_pool(name="sbp", bufs=3))
    expp = ctx.enter_context(tc.tile_pool(name="expp", bufs=6))
    outp = ctx.enter_context(tc.tile_pool(name="outp", bufs=3))
    rbp = ctx.enter_context(tc.tile_pool(name="rbp", bufs=2))
    psM = ctx.enter_context(tc.tile_pool(name="psM", bufs=2, space="PSUM"))
    psS = ctx.enter_context(tc.tile_pool(name="psS", bufs=2, space="PSUM"))
    psV = ctx.enter_context(tc.tile_pool(name="psV", bufs=2, space="PSUM"))
    drp = ctx.enter_context(tc.tile_pool(name="drp", bufs=2, space="DRAM"))

    # --- staging: small weights first, then xT strip-major so strip 0's
    # projection can start early while later strips stream in behind it.
    w_sb = {}
    xt_sb = singles.tile([128, 8, NT], BF16)

    def load_w(name, w):
        t = singles.tile([128, 8, CW], BF16, tag=f"w{name}", name=f"w_{name}")
        nc.gpsimd.dma_start(out=t, in_=w)
        w_sb[name] = t

    def load_xt(g):
        gs = g * QSTRIP
        nc.sync.dma_start(out=xt_sb[:, :, gs:gs + QSTRIP], in_=xt[:, g, :, :])

    load_w("q", wq)
    # strip 0 split across both DMA queues to cut time-to-first-matmul
    nc.sync.dma_start(out=xt_sb[:, 0:4, 0:QSTRIP], in_=xt[:, 0, 0:4, :])
    nc.gpsimd.dma_start(out=xt_sb[:, 4:8, 0:QSTRIP], in_=xt[:, 0, 4:8, :])
    load_xt(1)
    load_w("k", wk)
    load_w("v", wv)
    msk_sb = singles.tile([128, 2, 128], BF16)
    nc.gpsimd.dma_start(out=msk_sb, in_=msk)
    idn_sb = singles.tile([128, 128], BF16)
    nc.gpsimd.dma_start(out=idn_sb, in_=idn)
    wo_sb = singles.tile([128, D], BF16)
    nc.gpsimd.dma_start(out=wo_sb, in_=wo)
    for g in range(2, NSTRIP):
        load_xt(g)

    qt_sb = singles.tile([128, NT], BF16)
    kt_sb = singles.tile([128, NT], BF16)
    v_sb = singles.tile([128, B * HLOC * KT_TILES, HD + 1], BF16)
    nc.vector.memset(v_sb[:, :, HD:HD + 1], 1.0)

    # dedicated (non-pooled) avT tensors per strip: no WAR hazards
    avf = {g: singles.tile([128, QSTRIP], BF16, tag=f"avf{g}", name=f"avf{g}")
           for g in range(NSTRIP)}

    def proj_units(g):
        gs = g * QSTRIP
        st = {}

        def qk_mm(name, lo, hi, first, last, dst):
            def u():
                if first:
                    st[name] = psM.tile([128, QSTRIP], F32, tag="mm", name=f"ps_{name}")
                ps = st[name]
                for k in range(lo, hi):
                    nc.tensor.matmul(
                        ps, lhsT=w_sb[name][:, k, :],
                        rhs=xt_sb[:, k, gs:gs + QSTRIP],
                        start=(k == 0), stop=(k == 7))
                if last:
                    nc.vector.tensor_copy(dst[:, gs:gs + QSTRIP], ps)
            return u

        groups = {"q": [qk_mm("q", 0, 4, True, False, qt_sb),
                        qk_mm("q", 4, 8, False, True, qt_sb)],
                  "k": [qk_mm("k", 0, 4, True, False, kt_sb),
                        qk_mm("k", 4, 8, False, True, kt_sb)]}

        b, j = divmod(g, 4)

        def vt_mm(lo, hi, first, last):
            def u():
                if first:
                    st["v"] = psM.tile([128, QSTRIP], F32, tag="mm", name="ps_v")
                ps = st["v"]
                for k in range(lo, hi):
                    nc.tensor.matmul(
                        ps, lhsT=w_sb["v"][:, k, :],
                        rhs=xt_sb[:, k, gs:gs + QSTRIP],
                        start=(k == 0), stop=(k == 7))
                if last:
                    st["vt"] = sbp.tile([128, QSTRIP], BF16, tag="vt",
                                        name="vt_sb")
                    nc.vector.tensor_copy(st["vt"], ps)
            return u

        def v_tr():
            def u():
                tp = psM.tile([128, 4, 128], BF16, tag="mm", name="tp_ps")
                for tt in range(4):
                    nc.tensor.transpose(
                        tp[:, tt, :], st["vt"][:, tt * 128:(tt + 1) * 128],
                        idn_sb)
                for tt in range(4):
                    idx = b * HLOC * KT_TILES + 4 * j + tt
                    nc.vector.tensor_copy(
                        v_sb[:, idx:idx + KT_TILES + 1:KT_TILES, 0:HD],
                        tp[:, tt, :].rearrange("p (h d) -> p h d", h=2))
            return u

        groups["v"] = [vt_mm(0, 4, True, False), vt_mm(4, 8, False, True),
                       v_tr()]
        return groups

    def attn_units(g):
        """Per-kv-tile units: scores for both heads adjacent (row-tiled
        concurrency), one wide exp, triangular mask on diagonal tiles,
        then both heads' AV matmuls. Scores run 2 tiles ahead."""
        b, j = divmod(g, 4)
        T = 4 * (j + 1)
        st = {}

        def q0_of(t):
            return max(0, 128 * (t - 4 * j))

        def mk_sc(t):
            def u():
                sc = psS.tile([128, 2, QSTRIP], F32, tag="sc", name="sc_ps")
                st[t] = sc
                q0 = q0_of(t)
                for h in range(HLOC):
                    hp = h * HD
                    nc.tensor.matmul(
                        sc[:, h, q0:],
                        lhsT=kt_sb[hp:hp + HD,
                                   b * S + t * 128: b * S + (t + 1) * 128],
                        rhs=qt_sb[hp:hp + HD,
                                  b * S + j * QSTRIP + q0:
                                  b * S + (j + 1) * QSTRIP],
                        start=True, stop=True)
            return u

        def mk_ea(t):
            def u():
                sc = st.pop(t)
                q0 = q0_of(t)
                if t == 0:
                    st["av0"] = psV.tile([HD + 1, QSTRIP], F32, tag="av",
                                         name="av0_ps")
                    st["av1"] = psV.tile([HD + 1, QSTRIP], F32, tag="av",
                                         name="av1_ps")
                pexp = expp.tile([128, 2, QSTRIP], BF16, tag="pexp",
                                 name="pexp")
                nc.scalar.activation(
                    pexp[:, :, q0:], sc[:, :, q0:],
                    mybir.ActivationFunctionType.Exp, scale=0.125)
                if t >= 4 * j:  # diagonal block: triangular mask at q0
                    nc.vector.tensor_mul(
                        pexp[:, :, q0:q0 + 128], pexp[:, :, q0:q0 + 128],
                        msk_sb)
                for h in range(HLOC):
                    idx = (b * HLOC + h) * KT_TILES + t
                    nc.tensor.matmul(
                        st[f"av{h}"][:, q0:], lhsT=v_sb[:, idx, :],
                        rhs=pexp[:, h, q0:],
                        start=(t == 0), stop=(t == T - 1))
            return u

        def mk_norm():
            def u():
                # copy AV out of PSUM fast (releases the banks); denominator
                # row goes along, then a DRAM round trip broadcasts it
                av_sb = sbp.tile([HD + 1, 2, QSTRIP], F32, tag="avsb",
                                 name="av_sb")
                s_dr = drp.tile([2, QSTRIP], F32, tag="sdr")
                for h in range(HLOC):
                    nc.vector.tensor_copy(av_sb[:, h, :], st[f"av{h}"])
                    nc.sync.dma_start(out=s_dr[h:h + 1, :],
                                      in_=av_sb[HD:HD + 1, h, :])
                rb = rbp.tile([HD, 2, QSTRIP], F32, tag="rb")
                for h in range(HLOC):
                    nc.sync.dma_start(
                        out=rb[:, h, :],
                        in_=s_dr[h, :].partition_broadcast(HD))
                nc.vector.reciprocal_approx_fast(rb, rb)
                nc.vector.tensor_mul(avf[g][0:HD, :], av_sb[0:HD, 0, :],
                                     rb[:, 0, :])
                avh = sbp.tile([HD, QSTRIP], BF16, tag="avh")
                nc.vector.tensor_mul(avh, av_sb[0:HD, 1, :], rb[:, 1, :])
                nc.gpsimd.dma_start(out=avf[g][HD:2 * HD, :], in_=avh)
            return u

        units = [mk_sc(0), mk_sc(1)]
        for t in range(T):
            units.append(mk_ea(t))
            if t + 2 < T:
                units.append(mk_sc(t + 2))
        units.append(mk_norm())
        return units

    def out_units(g):
        gs = g * QSTRIP
        units = []

        def mk(tt):
            def u():
                ob = outp.tile([128, D], BF16, tag="ob")
                for n in range(2):
                    op_ps = psM.tile([128, 512], F32, tag="mm", name="op_ps")
                    nc.tensor.matmul(
                        op_ps, lhsT=avf[g][:, tt * 128:(tt + 1) * 128],
                        rhs=wo_sb[:, n * 512:(n + 1) * 512],
                        start=True, stop=True)
                    if n == 0:
                        nc.vector.tensor_copy(ob[:, n * 512:(n + 1) * 512],
                                              op_ps)
                    else:
                        nc.scalar.copy(ob[:, n * 512:(n + 1) * 512], op_ps)
                nc.sync.dma_start(
                    out=out[gs + tt * 128: gs + (tt + 1) * 128, :], in_=ob)
            return u
        for tt in range(4):
            units.append(mk(tt))
        return units

    # strip order: b0 ascending then b1 descending (short strip last).
    order = [0, 1, 2, 3, 7, 6, 5, 4]
    pu = {g: proj_units(g) for g in range(NSTRIP)}

    def pf(g, keys):
        return [u for k in keys for u in pu[g][k]]

    for u in pf(0, "qkv"):
        u()

    fill_sched = {
        0: lambda: pf(1, "qkv"),
        1: lambda: pf(2, "qkv") + out_units(0),
        2: lambda: pf(3, "qkv") + out_units(1),
        3: lambda: pf(7, "qkv") + pf(6, "kv") + pf(5, "kv") + pf(4, "kv")
                   + out_units(2),
        7: lambda: out_units(3) + pf(6, "q"),
        6: lambda: out_units(7) + pf(5, "q"),
        5: lambda: out_units(6) + pf(4, "q"),
        4: lambda: out_units(5),
    }
    for g in order:
        _interleave(attn_units(g), fill_sched[g]())
    for u in out_units(4):
        u()


_CACHED_NC = None


def build_module():
    global _CACHED_NC
    if _CACHED_NC is None:
        nc = bacc.Bacc("TRN2", debug=False)
        with tile.TileContext(nc) as tc:
            with ExitStack() as ctx:
                _build_kernel(ctx, tc)
        nc.compile()
        _CACHED_NC = nc
    return _CACHED_NC


def make_in_maps(x, Wq, Wk, Wv, Wo):
    x = np.asarray(x, np.float32)
    xT = x.reshape(NT, D).T.astype(NPBF16)          # [D, NT]
    # device layout [p, strip, k, col]: row d = k*128 + p
    xT = np.ascontiguousarray(
        xT.reshape(8, 128, NSTRIP, QSTRIP).transpose(1, 2, 0, 3))
    # triangular keep mask for the diagonal 128-block, duplicated per head
    i = np.arange(128)[:, None]
    c = np.arange(128)[None, :]
    tri = (c >= i).astype(NPBF16)
    msk = np.ascontiguousarray(
        np.stack([tri, tri], axis=1))               # [128, 2, 128]
    in_maps = []
    for core in range(NCORES):
        cs = slice(core * CW, (core + 1) * CW)
        def warr(W):  # [D, CW] -> [p, k, col] with d = k*128 + p
            a = np.asarray(W, np.float32)[:, cs].astype(NPBF16)
            return np.ascontiguousarray(
                a.reshape(8, 128, CW).transpose(1, 0, 2))
        in_maps.append({
            "xt": xT,
            "wq": warr(Wq),
            "wk": warr(Wk),
            "wv": warr(Wv),
            "wo": np.ascontiguousarray(np.asarray(Wo, np.float32)[cs, :]).astype(NPBF16),
            "msk": msk,
            "idn": np.eye(128, dtype=NPBF16),
        })
    return in_maps


def kernel(x, Wq, bq, Wk, bk, Wv, bv, Wo, bo):
    for b_ in (bq, bk, bv, bo):
        assert np.count_nonzero(np.asarray(b_)) == 0, "nonzero biases unsupported"
    nc = build_module()
    in_maps = make_in_maps(x, Wq, Wk, Wv, Wo)
    res = run_bass_kernel_spmd(nc, in_maps, core_ids=list(range(NCORES)))
    partials = [res.results[c]["out"] for c in range(NCORES)]
    total = np.sum(np.stack(partials, 0).astype(np.float32), axis=0)
    return total.reshape(B, S, D)
